# revision 20
# baseline (speedup 1.0000x reference)
"""Trainium2 Bass kernel for nn_BidirectionalAttention (B=16,H=4,T=256,N=2048,D=256).

Math (reference):
    Qr = rope2d(Q), Kr = rope2d(K)              # elementwise, per (t, n) angle
    scores = Qr @ Kr^T / sqrt(N)                # (B,H,T,T), no softmax
    out    = scores @ V                         # V (B,1,T,D) broadcasts over H

Kernel strategy (per core; B sharded 2-per-core across 8 cores):
  * Host staging does all layout work: Q and K are cast to bf16,
    feature-permuted to "pi order" (even features | odd features),
    transposed to [N, T] and laid out as the exact SBUF tile images
    ([128, ...] with one contiguous run per partition -> 1-line DMAs).
    This removes every on-device transpose and halves Q/K HBM traffic.
    scores are invariant under a shared feature permutation of Q and K.
  * In pi order the RoPE pair partner of feature row j is row j^1024:
    with the [N,T] tile split into lo (rows < 1024) and hi halves, the
    pair swap is just "read the other half-tile" - every elementwise op
    is a full-tile [128, 2048] contiguous bf16 op (DVE 2x mode).
  * RoPE per side: P1 = x*cT, P2 = swap(x)*sT (DVE 2x).  K-side combine
    Kr = P1k+P2k is split between GpSimd and DVE (tunable).  Q-side
    combine is FOLDED INTO mm1: each feature chunk contributes two
    accumulating matmuls (rhs = P1q chunk, then P2q chunk) into the
    same PSUM tile, so Qr is never materialized (sum is exact, in f32).
  * mm1: scoresT[s,t] accumulated over 16 chunks x 2 rope terms.
  * mm2: out[t,d] = sum_s scoresT[s][:,t_chunk] @ V[s]  (bf16)
  * 1/sqrt(N) folded into the tables (N^-1/4 each side).  V uploaded
    bf16 in tile layout; output written bf16 tile-layout, host restores.
"""

import math
import os
import numpy as np
import ml_dtypes
from contextlib import ExitStack

import concourse.bass as bass
import concourse.bacc as bacc_mod
import concourse.tile as tile
import concourse.mybir as mybir
from concourse.bass_utils import run_bass_kernel_spmd

bf16 = ml_dtypes.bfloat16

# problem shapes (hardcoded per contract)
B, H, T, N, D = 16, 4, 256, 2048, 256
GRID = 16
THETA = 10000.0
NCORES = 8
BS = B // NCORES          # batches per core
P = 128
NCH = N // P              # 16 feature chunks
TCH = T // P              # 2 token chunks
HC = NCH // 2             # 8 chunks per half
HW_ = HC * T              # 2048: free width of a half tile

LAST_RESULT = None        # BassKernelResults of the most recent run (for test.py)


def _rope_tables():
    """Host-precomputed cos/sin' tables in the transposed pi-order tile
    layout, one [128, 2048] tile per half; 1/sqrt(N) folded as N^-1/4."""
    half = N // 2
    inv_freq = (1.0 / THETA ** (np.arange(0, half, 2, dtype=np.float32) / np.float32(half))).astype(np.float32)
    pos = np.arange(GRID, dtype=np.float32)
    ph = pos[:, None] * inv_freq[None, :]                      # (16, 512)
    ph_h = np.broadcast_to(ph[:, None, :], (GRID, GRID, half // 2))
    ph_w = np.broadcast_to(ph[None, :, :], (GRID, GRID, half // 2))
    phases = np.concatenate([ph_h, ph_w, ph_h, ph_w], axis=-1).reshape(T, N)
    ang = np.mod(phases, np.float32(1.0)) * np.float32(2.0 * math.pi)
    alpha = np.float32(1.0 / math.sqrt(math.sqrt(N)))
    c = np.cos(ang) * alpha                                    # [t, n]
    s = np.sin(ang) * alpha
    # pi order halves: lo = even features, hi = odd.
    # s' multiplies the swapped partner x[j^1024]:
    #   lo rows (orig 2j):   Qr = x*c - swap(x)*s[2j]
    #   hi rows (orig 2j+1): Qr = x*c + swap(x)*s[2j+1]
    def tileize(x):                                            # [t, 1024] -> [128, 2048]
        xt = np.ascontiguousarray(x.T)                         # [j, t]
        return np.ascontiguousarray(
            xt.reshape(HC, P, T).transpose(1, 0, 2).reshape(P, HW_)).astype(bf16)
    ct = np.concatenate([tileize(c[:, 0::2]), tileize(c[:, 1::2])], axis=1)
    st = np.concatenate([tileize(-s[:, 0::2]), tileize(s[:, 1::2])], axis=1)
    return ct, st                                              # [128, 4096] each


def _build_nc():
    nc = bacc_mod.Bacc("TRN2", target_bir_lowering=False, debug=False)

    q_dram = nc.dram_tensor("QT", [BS, H, 2, P, HW_], mybir.dt.bfloat16, kind="ExternalInput").ap()
    k_dram = nc.dram_tensor("KT", [BS, H, 2, P, HW_], mybir.dt.bfloat16, kind="ExternalInput").ap()
    v_dram = nc.dram_tensor("V", [BS, P, TCH * D], mybir.dt.bfloat16, kind="ExternalInput").ap()
    c_dram = nc.dram_tensor("CT", [P, 2 * HW_], mybir.dt.bfloat16, kind="ExternalInput").ap()
    s_dram = nc.dram_tensor("ST", [P, 2 * HW_], mybir.dt.bfloat16, kind="ExternalInput").ap()
    o_dram = nc.dram_tensor("O", [BS, H, P, TCH * D], mybir.dt.bfloat16, kind="ExternalOutput").ap()

    with tile.TileContext(nc) as tc, ExitStack() as ctx:
        const_pool = ctx.enter_context(tc.tile_pool(name="const", bufs=1))
        qk_pool = ctx.enter_context(tc.tile_pool(name="qk", bufs=3))
        v_pool = ctx.enter_context(tc.tile_pool(name="vp", bufs=2))
        rp_pool = ctx.enter_context(tc.tile_pool(name="rp", bufs=2))
        kr_pool = ctx.enter_context(tc.tile_pool(name="kr", bufs=2))
        sc_pool = ctx.enter_context(tc.tile_pool(name="scp", bufs=2))
        out_pool = ctx.enter_context(tc.tile_pool(name="outp", bufs=2))
        ps_sc = ctx.enter_context(tc.tile_pool(name="pssc", bufs=2, space="PSUM"))
        ps_out = ctx.enter_context(tc.tile_pool(name="psout", bufs=2, space="PSUM"))

        ct = const_pool.tile([P, 2 * HW_], mybir.dt.bfloat16)
        st = const_pool.tile([P, 2 * HW_], mybir.dt.bfloat16)
        nc.sync.dma_start(ct[:], c_dram)
        nc.scalar.dma_start(st[:], s_dram)
        # absorb the table-DMA waits into DVE engine order once so the
        # per-(b,h) multiplies never need more than one sem wait
        scr1 = const_pool.tile([1, 8], mybir.dt.bfloat16)
        scr2 = const_pool.tile([1, 8], mybir.dt.bfloat16)
        nc.vector.tensor_copy(scr1[:], ct[0:1, 0:8])
        nc.vector.tensor_copy(scr2[:], st[0:1, 0:8])

        for b in range(BS):
            v_bf = v_pool.tile([P, TCH * D], mybir.dt.bfloat16, tag="vbf")
            nc.sync.dma_start(v_bf[:], v_dram[b])
            for h in range(H):
                halves = {}
                for (name, dram, eng) in (("q", q_dram, nc.sync), ("k", k_dram, nc.scalar)):
                    for hf in range(2):
                        tl = qk_pool.tile([P, HW_], mybir.dt.bfloat16, tag=f"{name}t{hf}")
                        eng.dma_start(tl[:], dram[b, h, hf])
                        halves[(name, hf)] = tl

                # RoPE products: all full-tile [128, 2048] contiguous
                # (DVE 2x bf16).  P2 half hf reads x's other half.
                # The K chain (muls + combine) stays on DVE so kr never
                # waits on a cross-engine hop; GpSimd takes the hf=1
                # Q-products, which are pure-parallel work.
                # All vector work on DVE, grouped per half so the inputs
                # of mm1's chunk group hf complete together: PE's idle
                # gaps stay under the HAM re-throttle window.
                pp = {}
                kr = {}
                for hf in range(2):
                    sl = slice(hf * HW_, (hf + 1) * HW_)
                    p1k = rp_pool.tile([P, HW_], mybir.dt.bfloat16, tag=f"p1k{hf}")
                    p2k = rp_pool.tile([P, HW_], mybir.dt.bfloat16, tag=f"p2k{hf}")
                    nc.vector.tensor_mul(p1k[:], halves[("k", hf)][:], ct[:, sl])
                    nc.vector.tensor_mul(p2k[:], halves[("k", 1 - hf)][:], st[:, sl])
                    p1q = rp_pool.tile([P, HW_], mybir.dt.bfloat16, tag=f"p1q{hf}")
                    p2q = rp_pool.tile([P, HW_], mybir.dt.bfloat16, tag=f"p2q{hf}")
                    nc.vector.tensor_mul(p1q[:], halves[("q", hf)][:], ct[:, sl])
                    nc.vector.tensor_mul(p2q[:], halves[("q", 1 - hf)][:], st[:, sl])
                    # hf=1's combine rides on the otherwise-idle GpSimd;
                    # its ~5us latency is hidden by mm1's c<8 group.
                    krt = kr_pool.tile([P, HW_], mybir.dt.bfloat16, tag=f"kr{hf}")
                    eng = nc.gpsimd if hf == 1 else nc.vector
                    eng.tensor_add(krt[:], p1k[:], p2k[:])
                    kr[hf] = krt
                    pp[("q", hf)] = (p1q, p2q)

                # mm1: scoresT[s,t] = sum_c Kr[c]^T-slice @ (P1q[c] + P2q[c])
                sc_ps = []
                for sch in range(TCH):
                    sc_tile = ps_sc.tile([P, T], mybir.dt.float32, tag=f"scps{sch}")
                    sc_ps.append(sc_tile)
                for c in range(NCH):
                    hf, cc = divmod(c, HC)
                    p1q, p2q = pp[("q", hf)]
                    for sch in range(TCH):
                        lhsT = kr[hf][:, cc * T + sch * P: cc * T + (sch + 1) * P]
                        nc.tensor.matmul(sc_ps[sch][:], lhsT,
                                         p1q[:, cc * T:(cc + 1) * T],
                                         start=(c == 0), stop=False)
                        nc.tensor.matmul(sc_ps[sch][:], lhsT,
                                         p2q[:, cc * T:(cc + 1) * T],
                                         start=False, stop=(c == NCH - 1))
                sc_sb = sc_pool.tile([P, TCH * T], mybir.dt.bfloat16, tag="scsb")
                for sch in range(TCH):
                    nc.scalar.copy(sc_sb[:, sch * T:(sch + 1) * T], sc_ps[sch][:])

                # mm2: out[t_chunk] [128t, 256d] = sum_s scoresT[s][:,t_chunk] @ V[s]
                o_ps = ps_out.tile([P, TCH * D], mybir.dt.float32, tag="ops")
                for tch in range(TCH):
                    for sch in range(TCH):
                        lhsT = sc_sb[:, sch * T + tch * P: sch * T + (tch + 1) * P]
                        rhs = v_bf[:, sch * D:(sch + 1) * D]
                        nc.tensor.matmul(o_ps[:, tch * D:(tch + 1) * D], lhsT, rhs,
                                         start=(sch == 0), stop=(sch == TCH - 1))
                o_sb = out_pool.tile([P, TCH * D], mybir.dt.bfloat16, tag="osb")
                nc.scalar.copy(o_sb[:], o_ps[:])
                nc.scalar.dma_start(o_dram[b, h], o_sb[:])
    return nc


_NC_CACHE = None


def _stage_qk(x):
    """[B,H,T,N] f32 -> bf16, pi order, [B,H,2,128,2048] half-tile images."""
    xb = x.astype(bf16)
    xt = xb.transpose(0, 1, 3, 2)                 # [B,H,N,T] view
    out = np.empty((x.shape[0], x.shape[1], 2, P, HW_), dtype=bf16)
    for hf, sl in ((0, xt[:, :, 0::2, :]), (1, xt[:, :, 1::2, :])):
        # [B,H,1024,T] -> [B,H,8,128,T] -> [B,H,128,8,T] -> [B,H,128,2048]
        out[:, :, hf] = np.ascontiguousarray(
            sl.reshape(x.shape[0], x.shape[1], HC, P, T).transpose(0, 1, 3, 2, 4)
        ).reshape(x.shape[0], x.shape[1], P, HW_)
    return out


def _stage_v(V):
    """[B,1,T,D] f32 -> bf16 tile image [B,128,TCH*D]."""
    vb = V.astype(bf16)[:, 0]                     # [B,T,D]
    return np.ascontiguousarray(
        vb.reshape(B, TCH, P, D).transpose(0, 2, 1, 3)).reshape(B, P, TCH * D)


def _unstage_o(o):
    """[B,H,128,TCH*D] bf16 tile image -> [B,H,T,D] f32."""
    return np.ascontiguousarray(
        o.reshape(B, H, P, TCH, D).transpose(0, 1, 3, 2, 4)
    ).reshape(B, H, T, D).astype(np.float32)


def kernel(Q, K, V):
    global _NC_CACHE, LAST_RESULT
    Q = np.asarray(Q, dtype=np.float32)
    K = np.asarray(K, dtype=np.float32)
    V = np.asarray(V, dtype=np.float32)
    assert Q.shape == (B, H, T, N) and K.shape == (B, H, T, N) and V.shape == (B, 1, T, D)

    if _NC_CACHE is None:
        _NC_CACHE = _build_nc()
        _NC_CACHE.compile()
    nc = _NC_CACHE
    ct, st = _rope_tables()
    QT = _stage_qk(Q)
    KT = _stage_qk(K)
    Vb = _stage_v(V)

    in_maps = []
    for c in range(NCORES):
        sl = slice(c * BS, (c + 1) * BS)
        in_maps.append({
            "QT": np.ascontiguousarray(QT[sl]),
            "KT": np.ascontiguousarray(KT[sl]),
            "V": np.ascontiguousarray(Vb[sl]),
            "CT": ct, "ST": st,
        })

    trace = bool(os.environ.get("BASS_KERNEL_TRACE"))
    res = run_bass_kernel_spmd(nc, in_maps, list(range(NCORES)), trace=trace,
                               trace_cores=[0] if trace else None)
    LAST_RESULT = res
    o = np.concatenate([res.results[c]["O"] for c in range(NCORES)], axis=0)
    return _unstage_o(o)


# revision 21
# speedup vs baseline: 1.1814x; 1.1814x over previous
"""Trainium2 Bass kernel for nn_BidirectionalAttention (B=16,H=4,T=256,N=2048,D=256).

Math (reference):
    Qr = rope2d(Q), Kr = rope2d(K)              # elementwise, per (t, n) angle
    scores = Qr @ Kr^T / sqrt(N)                # (B,H,T,T), no softmax
    out    = scores @ V                         # V (B,1,T,D) broadcasts over H

Kernel strategy (per core; B sharded 2-per-core across 8 cores):
  * Host staging does all layout work: Q and K are cast to bf16,
    feature-permuted to "pi order" (even features | odd features),
    transposed to [N, T] and laid out as the exact SBUF tile images
    ([128, ...] with one contiguous run per partition -> 1-line DMAs).
    This removes every on-device transpose and halves Q/K HBM traffic.
    scores are invariant under a shared feature permutation of Q and K.
  * In pi order the RoPE pair partner of feature row j is row j^1024:
    with the [N,T] tile split into lo (rows < 1024) and hi halves, the
    pair swap is just "read the other half-tile" - every elementwise op
    is a full-tile [128, 2048] contiguous bf16 op (DVE 2x mode).
  * RoPE per side: P1 = x*cT, P2 = swap(x)*sT (DVE 2x).  K-side combine
    Kr = P1k+P2k is split between GpSimd and DVE (tunable).  Q-side
    combine is FOLDED INTO mm1: each feature chunk contributes two
    accumulating matmuls (rhs = P1q chunk, then P2q chunk) into the
    same PSUM tile, so Qr is never materialized (sum is exact, in f32).
  * mm1: scoresT[s,t] accumulated over 16 chunks x 2 rope terms.
  * mm2: out[t,d] = sum_s scoresT[s][:,t_chunk] @ V[s]  (bf16)
  * 1/sqrt(N) folded into the tables (N^-1/4 each side).  V uploaded
    bf16 in tile layout; output written bf16 tile-layout, host restores.
"""

import math
import os
import numpy as np
import ml_dtypes
from contextlib import ExitStack

import concourse.bass as bass
import concourse.bacc as bacc_mod
import concourse.tile as tile
import concourse.mybir as mybir
from concourse.bass_utils import run_bass_kernel_spmd

bf16 = ml_dtypes.bfloat16

# problem shapes (hardcoded per contract)
B, H, T, N, D = 16, 4, 256, 2048, 256
GRID = 16
THETA = 10000.0
NCORES = 8
BS = B // NCORES          # batches per core
P = 128
NCH = N // P              # 16 feature chunks
TCH = T // P              # 2 token chunks
HC = NCH // 2             # 8 chunks per half
HW_ = HC * T              # 2048: free width of a half tile

LAST_RESULT = None        # BassKernelResults of the most recent run (for test.py)


def _rope_tables():
    """Host-precomputed cos/sin' tables in the transposed pi-order tile
    layout, one [128, 2048] tile per half; 1/sqrt(N) folded as N^-1/4."""
    half = N // 2
    inv_freq = (1.0 / THETA ** (np.arange(0, half, 2, dtype=np.float32) / np.float32(half))).astype(np.float32)
    pos = np.arange(GRID, dtype=np.float32)
    ph = pos[:, None] * inv_freq[None, :]                      # (16, 512)
    ph_h = np.broadcast_to(ph[:, None, :], (GRID, GRID, half // 2))
    ph_w = np.broadcast_to(ph[None, :, :], (GRID, GRID, half // 2))
    phases = np.concatenate([ph_h, ph_w, ph_h, ph_w], axis=-1).reshape(T, N)
    ang = np.mod(phases, np.float32(1.0)) * np.float32(2.0 * math.pi)
    alpha = np.float32(1.0 / math.sqrt(math.sqrt(N)))
    c = np.cos(ang) * alpha                                    # [t, n]
    s = np.sin(ang) * alpha
    # pi order halves: lo = even features, hi = odd.
    # s' multiplies the swapped partner x[j^1024]:
    #   lo rows (orig 2j):   Qr = x*c - swap(x)*s[2j]
    #   hi rows (orig 2j+1): Qr = x*c + swap(x)*s[2j+1]
    def tileize(x):                                            # [t, 1024] -> [128, 2048]
        xt = np.ascontiguousarray(x.T)                         # [j, t]
        return np.ascontiguousarray(
            xt.reshape(HC, P, T).transpose(1, 0, 2).reshape(P, HW_)).astype(bf16)
    ct = np.concatenate([tileize(c[:, 0::2]), tileize(c[:, 1::2])], axis=1)
    st = np.concatenate([tileize(-s[:, 0::2]), tileize(s[:, 1::2])], axis=1)
    return ct, st                                              # [128, 4096] each


def _build_nc():
    nc = bacc_mod.Bacc("TRN2", target_bir_lowering=False, debug=False)

    q_dram = nc.dram_tensor("QT", [BS, H, 2, P, HW_], mybir.dt.bfloat16, kind="ExternalInput").ap()
    k_dram = nc.dram_tensor("KT", [BS, H, 2, P, HW_], mybir.dt.bfloat16, kind="ExternalInput").ap()
    v_dram = nc.dram_tensor("V", [BS, P, TCH * D], mybir.dt.bfloat16, kind="ExternalInput").ap()
    c_dram = nc.dram_tensor("CT", [P, 2 * HW_], mybir.dt.bfloat16, kind="ExternalInput").ap()
    s_dram = nc.dram_tensor("ST", [P, 2 * HW_], mybir.dt.bfloat16, kind="ExternalInput").ap()
    o_dram = nc.dram_tensor("O", [BS, H, P, TCH * D], mybir.dt.bfloat16, kind="ExternalOutput").ap()

    with tile.TileContext(nc) as tc, ExitStack() as ctx:
        const_pool = ctx.enter_context(tc.tile_pool(name="const", bufs=1))
        qk_pool = ctx.enter_context(tc.tile_pool(name="qk", bufs=3))
        v_pool = ctx.enter_context(tc.tile_pool(name="vp", bufs=2))
        rp_pool = ctx.enter_context(tc.tile_pool(name="rp", bufs=2))
        kr_pool = ctx.enter_context(tc.tile_pool(name="kr", bufs=2))
        sc_pool = ctx.enter_context(tc.tile_pool(name="scp", bufs=2))
        out_pool = ctx.enter_context(tc.tile_pool(name="outp", bufs=2))
        ps_sc = ctx.enter_context(tc.tile_pool(name="pssc", bufs=2, space="PSUM"))
        ps_out = ctx.enter_context(tc.tile_pool(name="psout", bufs=2, space="PSUM"))

        ct = const_pool.tile([P, 2 * HW_], mybir.dt.bfloat16)
        st = const_pool.tile([P, 2 * HW_], mybir.dt.bfloat16)
        nc.sync.dma_start(ct[:], c_dram)
        nc.scalar.dma_start(st[:], s_dram)
        # absorb the table-DMA waits into DVE engine order once so the
        # per-(b,h) multiplies never need more than one sem wait
        scr1 = const_pool.tile([1, 8], mybir.dt.bfloat16)
        scr2 = const_pool.tile([1, 8], mybir.dt.bfloat16)
        nc.vector.tensor_copy(scr1[:], ct[0:1, 0:8])
        nc.vector.tensor_copy(scr2[:], st[0:1, 0:8])

        for b in range(BS):
            v_bf = v_pool.tile([P, TCH * D], mybir.dt.bfloat16, tag="vbf")
            nc.sync.dma_start(v_bf[:], v_dram[b])
            for h in range(H):
                halves = {}
                for (name, dram, eng) in (("q", q_dram, nc.sync), ("k", k_dram, nc.scalar)):
                    for hf in range(2):
                        tl = qk_pool.tile([P, HW_], mybir.dt.bfloat16, tag=f"{name}t{hf}")
                        eng.dma_start(tl[:], dram[b, h, hf])
                        halves[(name, hf)] = tl

                # RoPE products: all full-tile [128, 2048] contiguous
                # (DVE 2x bf16).  P2 half hf reads x's other half.
                # The K chain (muls + combine) stays on DVE so kr never
                # waits on a cross-engine hop; GpSimd takes the hf=1
                # Q-products, which are pure-parallel work.
                # All vector work on DVE, grouped per half so the inputs
                # of mm1's chunk group hf complete together: PE's idle
                # gaps stay under the HAM re-throttle window.
                pp = {}
                kr = {}
                for hf in range(2):
                    sl = slice(hf * HW_, (hf + 1) * HW_)
                    p1k = rp_pool.tile([P, HW_], mybir.dt.bfloat16, tag=f"p1k{hf}")
                    p2k = rp_pool.tile([P, HW_], mybir.dt.bfloat16, tag=f"p2k{hf}")
                    nc.vector.tensor_mul(p1k[:], halves[("k", hf)][:], ct[:, sl])
                    nc.vector.tensor_mul(p2k[:], halves[("k", 1 - hf)][:], st[:, sl])
                    p1q = rp_pool.tile([P, HW_], mybir.dt.bfloat16, tag=f"p1q{hf}")
                    p2q = rp_pool.tile([P, HW_], mybir.dt.bfloat16, tag=f"p2q{hf}")
                    nc.vector.tensor_mul(p1q[:], halves[("q", hf)][:], ct[:, sl])
                    nc.vector.tensor_mul(p2q[:], halves[("q", 1 - hf)][:], st[:, sl])
                    krt = kr_pool.tile([P, HW_], mybir.dt.bfloat16, tag=f"kr{hf}")
                    nc.vector.tensor_add(krt[:], p1k[:], p2k[:])
                    kr[hf] = krt
                    pp[("q", hf)] = (p1q, p2q)

                # mm1: scoresT[s,t] = sum_c Kr[c]^T-slice @ (P1q[c] + P2q[c])
                sc_ps = []
                for sch in range(TCH):
                    sc_tile = ps_sc.tile([P, T], mybir.dt.float32, tag=f"scps{sch}")
                    sc_ps.append(sc_tile)
                for c in range(NCH):
                    hf, cc = divmod(c, HC)
                    p1q, p2q = pp[("q", hf)]
                    for sch in range(TCH):
                        lhsT = kr[hf][:, cc * T + sch * P: cc * T + (sch + 1) * P]
                        nc.tensor.matmul(sc_ps[sch][:], lhsT,
                                         p1q[:, cc * T:(cc + 1) * T],
                                         start=(c == 0), stop=False)
                        nc.tensor.matmul(sc_ps[sch][:], lhsT,
                                         p2q[:, cc * T:(cc + 1) * T],
                                         start=False, stop=(c == NCH - 1))
                sc_sb = sc_pool.tile([P, TCH * T], mybir.dt.bfloat16, tag="scsb")
                for sch in range(TCH):
                    nc.scalar.copy(sc_sb[:, sch * T:(sch + 1) * T], sc_ps[sch][:])

                # mm2: out[t_chunk] [128t, 256d] = sum_s scoresT[s][:,t_chunk] @ V[s]
                o_ps = ps_out.tile([P, TCH * D], mybir.dt.float32, tag="ops")
                for tch in range(TCH):
                    for sch in range(TCH):
                        lhsT = sc_sb[:, sch * T + tch * P: sch * T + (tch + 1) * P]
                        rhs = v_bf[:, sch * D:(sch + 1) * D]
                        nc.tensor.matmul(o_ps[:, tch * D:(tch + 1) * D], lhsT, rhs,
                                         start=(sch == 0), stop=(sch == TCH - 1))
                o_sb = out_pool.tile([P, TCH * D], mybir.dt.bfloat16, tag="osb")
                nc.scalar.copy(o_sb[:], o_ps[:])
                nc.scalar.dma_start(o_dram[b, h], o_sb[:])
    return nc


_NC_CACHE = None


def _stage_qk(x):
    """[B,H,T,N] f32 -> bf16, pi order, [B,H,2,128,2048] half-tile images."""
    xb = x.astype(bf16)
    xt = xb.transpose(0, 1, 3, 2)                 # [B,H,N,T] view
    out = np.empty((x.shape[0], x.shape[1], 2, P, HW_), dtype=bf16)
    for hf, sl in ((0, xt[:, :, 0::2, :]), (1, xt[:, :, 1::2, :])):
        # [B,H,1024,T] -> [B,H,8,128,T] -> [B,H,128,8,T] -> [B,H,128,2048]
        out[:, :, hf] = np.ascontiguousarray(
            sl.reshape(x.shape[0], x.shape[1], HC, P, T).transpose(0, 1, 3, 2, 4)
        ).reshape(x.shape[0], x.shape[1], P, HW_)
    return out


def _stage_v(V):
    """[B,1,T,D] f32 -> bf16 tile image [B,128,TCH*D]."""
    vb = V.astype(bf16)[:, 0]                     # [B,T,D]
    return np.ascontiguousarray(
        vb.reshape(B, TCH, P, D).transpose(0, 2, 1, 3)).reshape(B, P, TCH * D)


def _unstage_o(o):
    """[B,H,128,TCH*D] bf16 tile image -> [B,H,T,D] f32."""
    return np.ascontiguousarray(
        o.reshape(B, H, P, TCH, D).transpose(0, 1, 3, 2, 4)
    ).reshape(B, H, T, D).astype(np.float32)


def kernel(Q, K, V):
    global _NC_CACHE, LAST_RESULT
    Q = np.asarray(Q, dtype=np.float32)
    K = np.asarray(K, dtype=np.float32)
    V = np.asarray(V, dtype=np.float32)
    assert Q.shape == (B, H, T, N) and K.shape == (B, H, T, N) and V.shape == (B, 1, T, D)

    if _NC_CACHE is None:
        _NC_CACHE = _build_nc()
        _NC_CACHE.compile()
    nc = _NC_CACHE
    ct, st = _rope_tables()
    QT = _stage_qk(Q)
    KT = _stage_qk(K)
    Vb = _stage_v(V)

    in_maps = []
    for c in range(NCORES):
        sl = slice(c * BS, (c + 1) * BS)
        in_maps.append({
            "QT": np.ascontiguousarray(QT[sl]),
            "KT": np.ascontiguousarray(KT[sl]),
            "V": np.ascontiguousarray(Vb[sl]),
            "CT": ct, "ST": st,
        })

    trace = bool(os.environ.get("BASS_KERNEL_TRACE"))
    res = run_bass_kernel_spmd(nc, in_maps, list(range(NCORES)), trace=trace,
                               trace_cores=[0] if trace else None)
    LAST_RESULT = res
    o = np.concatenate([res.results[c]["O"] for c in range(NCORES)], axis=0)
    return _unstage_o(o)


# revision 24
# speedup vs baseline: 1.2057x; 1.0206x over previous
"""Trainium2 Bass kernel for nn_BidirectionalAttention (B=16,H=4,T=256,N=2048,D=256).

Math (reference):
    Qr = rope2d(Q), Kr = rope2d(K)              # elementwise, per (t, n) angle
    scores = Qr @ Kr^T / sqrt(N)                # (B,H,T,T), no softmax
    out    = scores @ V                         # V (B,1,T,D) broadcasts over H

Kernel strategy (per core; B sharded 2-per-core across 8 cores):
  * Host staging does all layout work: Q and K are cast to bf16,
    feature-permuted to "pi order" (even features | odd features),
    transposed to [N, T] and laid out as the exact SBUF tile images
    ([128, ...] with one contiguous run per partition -> 1-line DMAs).
    This removes every on-device transpose and halves Q/K HBM traffic.
    scores are invariant under a shared feature permutation of Q and K.
  * In pi order the RoPE pair partner of feature row j is row j^1024:
    with the [N,T] tile split into lo (rows < 1024) and hi halves, the
    pair swap is just "read the other half-tile" - every elementwise op
    is a full-tile [128, 2048] contiguous bf16 op (DVE 2x mode).
  * RoPE per side: P1 = x*cT, P2 = swap(x)*sT (DVE 2x).  K-side combine
    Kr = P1k+P2k is split between GpSimd and DVE (tunable).  Q-side
    combine is FOLDED INTO mm1: each feature chunk contributes two
    accumulating matmuls (rhs = P1q chunk, then P2q chunk) into the
    same PSUM tile, so Qr is never materialized (sum is exact, in f32).
  * mm1: scoresT[s,t] accumulated over 16 chunks x 2 rope terms.
  * mm2: out[t,d] = sum_s scoresT[s][:,t_chunk] @ V[s]  (bf16)
  * 1/sqrt(N) folded into the tables (N^-1/4 each side).  V uploaded
    bf16 in tile layout; output written bf16 tile-layout, host restores.
"""

import math
import os
import numpy as np
import ml_dtypes
from contextlib import ExitStack

import concourse.bass as bass
import concourse.bacc as bacc_mod
import concourse.tile as tile
import concourse.mybir as mybir
from concourse.bass_utils import run_bass_kernel_spmd

bf16 = ml_dtypes.bfloat16

# problem shapes (hardcoded per contract)
B, H, T, N, D = 16, 4, 256, 2048, 256
GRID = 16
THETA = 10000.0
NCORES = 8
BS = B // NCORES          # batches per core
P = 128
NCH = N // P              # 16 feature chunks
TCH = T // P              # 2 token chunks
HC = NCH // 2             # 8 chunks per half
HW_ = HC * T              # 2048: free width of a half tile

LAST_RESULT = None        # BassKernelResults of the most recent run (for test.py)


def _rope_tables():
    """Host-precomputed cos/sin' tables in the transposed pi-order tile
    layout, one [128, 2048] tile per half; 1/sqrt(N) folded as N^-1/4."""
    half = N // 2
    inv_freq = (1.0 / THETA ** (np.arange(0, half, 2, dtype=np.float32) / np.float32(half))).astype(np.float32)
    pos = np.arange(GRID, dtype=np.float32)
    ph = pos[:, None] * inv_freq[None, :]                      # (16, 512)
    ph_h = np.broadcast_to(ph[:, None, :], (GRID, GRID, half // 2))
    ph_w = np.broadcast_to(ph[None, :, :], (GRID, GRID, half // 2))
    phases = np.concatenate([ph_h, ph_w, ph_h, ph_w], axis=-1).reshape(T, N)
    ang = np.mod(phases, np.float32(1.0)) * np.float32(2.0 * math.pi)
    alpha = np.float32(1.0 / math.sqrt(math.sqrt(N)))
    c = np.cos(ang) * alpha                                    # [t, n]
    s = np.sin(ang) * alpha
    # pi order halves: lo = even features, hi = odd.
    # s' multiplies the swapped partner x[j^1024]:
    #   lo rows (orig 2j):   Qr = x*c - swap(x)*s[2j]
    #   hi rows (orig 2j+1): Qr = x*c + swap(x)*s[2j+1]
    def tileize(x):                                            # [t, 1024] -> [128, 2048]
        xt = np.ascontiguousarray(x.T)                         # [j, t]
        return np.ascontiguousarray(
            xt.reshape(HC, P, T).transpose(1, 0, 2).reshape(P, HW_)).astype(bf16)
    ct = np.concatenate([tileize(c[:, 0::2]), tileize(c[:, 1::2])], axis=1)
    st = np.concatenate([tileize(-s[:, 0::2]), tileize(s[:, 1::2])], axis=1)
    return ct, st                                              # [128, 4096] each


def _build_nc():
    nc = bacc_mod.Bacc("TRN2", target_bir_lowering=False, debug=False)

    q_dram = nc.dram_tensor("QT", [BS, H, 2, P, HW_], mybir.dt.bfloat16, kind="ExternalInput").ap()
    k_dram = nc.dram_tensor("KT", [BS, H, 2, P, HW_], mybir.dt.bfloat16, kind="ExternalInput").ap()
    v_dram = nc.dram_tensor("V", [BS, P, TCH * D], mybir.dt.bfloat16, kind="ExternalInput").ap()
    c_dram = nc.dram_tensor("CT", [P, 2 * HW_], mybir.dt.bfloat16, kind="ExternalInput").ap()
    s_dram = nc.dram_tensor("ST", [P, 2 * HW_], mybir.dt.bfloat16, kind="ExternalInput").ap()
    o_dram = nc.dram_tensor("O", [BS, H, P, TCH * D], mybir.dt.bfloat16, kind="ExternalOutput").ap()

    with tile.TileContext(nc) as tc, ExitStack() as ctx:
        const_pool = ctx.enter_context(tc.tile_pool(name="const", bufs=1))
        qk_pool = ctx.enter_context(tc.tile_pool(name="qk", bufs=3))
        v_pool = ctx.enter_context(tc.tile_pool(name="vp", bufs=2))
        rp_pool = ctx.enter_context(tc.tile_pool(name="rp", bufs=2))
        kr_pool = ctx.enter_context(tc.tile_pool(name="kr", bufs=2))
        sc_pool = ctx.enter_context(tc.tile_pool(name="scp", bufs=2))
        out_pool = ctx.enter_context(tc.tile_pool(name="outp", bufs=2))
        ps_sc = ctx.enter_context(tc.tile_pool(name="pssc", bufs=2, space="PSUM"))
        ps_out = ctx.enter_context(tc.tile_pool(name="psout", bufs=2, space="PSUM"))

        # Tables as half-tiles: the hf=0 halves load first (small, fast)
        # so the first pair's muls start early; the hf=1 halves stream in
        # behind the first Q/K tiles and are absorbed into DVE order only
        # after the first hf=0 mul group (an upfront absorb would block
        # the DVE FIFO on the late half).
        ctt, stt = [], []
        for hf in range(2):
            c_t = const_pool.tile([P, HW_], mybir.dt.bfloat16, tag=f"ctt{hf}")
            s_t = const_pool.tile([P, HW_], mybir.dt.bfloat16, tag=f"stt{hf}")
            ctt.append(c_t)
            stt.append(s_t)
        nc.sync.dma_start(ctt[0][:], c_dram[:, 0:HW_])
        nc.scalar.dma_start(stt[0][:], s_dram[:, 0:HW_])
        scr1 = const_pool.tile([1, 8], mybir.dt.bfloat16)
        scr2 = const_pool.tile([1, 8], mybir.dt.bfloat16)
        nc.vector.tensor_copy(scr1[:], ctt[0][0:1, 0:8])
        nc.vector.tensor_copy(scr2[:], stt[0][0:1, 0:8])

        for b in range(BS):
            v_bf = v_pool.tile([P, TCH * D], mybir.dt.bfloat16, tag="vbf")
            nc.sync.dma_start(v_bf[:], v_dram[b])
            for h in range(H):
                halves = {}
                for (name, dram, eng) in (("q", q_dram, nc.sync), ("k", k_dram, nc.scalar)):
                    for hf in range(2):
                        tl = qk_pool.tile([P, HW_], mybir.dt.bfloat16, tag=f"{name}t{hf}")
                        eng.dma_start(tl[:], dram[b, h, hf])
                        halves[(name, hf)] = tl
                if b == 0 and h == 0:
                    nc.sync.dma_start(ctt[1][:], c_dram[:, HW_:])
                    nc.scalar.dma_start(stt[1][:], s_dram[:, HW_:])

                # RoPE products: all full-tile [128, 2048] contiguous
                # (DVE 2x bf16).  P2 half hf reads x's other half.
                # The K chain (muls + combine) stays on DVE so kr never
                # waits on a cross-engine hop; GpSimd takes the hf=1
                # Q-products, which are pure-parallel work.
                # All vector work on DVE, grouped per half so the inputs
                # of mm1's chunk group hf complete together: PE's idle
                # gaps stay under the HAM re-throttle window.
                pp = {}
                kr = {}
                for hf in range(2):
                    if b == 0 and h == 0 and hf == 1:
                        # absorb the late table halves now that the hf=0
                        # products are already in flight
                        scr3 = const_pool.tile([1, 8], mybir.dt.bfloat16)
                        scr4 = const_pool.tile([1, 8], mybir.dt.bfloat16)
                        nc.vector.tensor_copy(scr3[:], ctt[1][0:1, 0:8])
                        nc.vector.tensor_copy(scr4[:], stt[1][0:1, 0:8])
                    p1k = rp_pool.tile([P, HW_], mybir.dt.bfloat16, tag=f"p1k{hf}")
                    p2k = rp_pool.tile([P, HW_], mybir.dt.bfloat16, tag=f"p2k{hf}")
                    nc.vector.tensor_mul(p1k[:], halves[("k", hf)][:], ctt[hf][:])
                    nc.vector.tensor_mul(p2k[:], halves[("k", 1 - hf)][:], stt[hf][:])
                    p1q = rp_pool.tile([P, HW_], mybir.dt.bfloat16, tag=f"p1q{hf}")
                    p2q = rp_pool.tile([P, HW_], mybir.dt.bfloat16, tag=f"p2q{hf}")
                    nc.vector.tensor_mul(p1q[:], halves[("q", hf)][:], ctt[hf][:])
                    nc.vector.tensor_mul(p2q[:], halves[("q", 1 - hf)][:], stt[hf][:])
                    krt = kr_pool.tile([P, HW_], mybir.dt.bfloat16, tag=f"kr{hf}")
                    nc.vector.tensor_add(krt[:], p1k[:], p2k[:])
                    kr[hf] = krt
                    pp[("q", hf)] = (p1q, p2q)

                # mm1: scoresT[s,t] = sum_c Kr[c]^T-slice @ (P1q[c] + P2q[c])
                sc_ps = []
                for sch in range(TCH):
                    sc_tile = ps_sc.tile([P, T], mybir.dt.float32, tag=f"scps{sch}")
                    sc_ps.append(sc_tile)
                for c in range(NCH):
                    hf, cc = divmod(c, HC)
                    p1q, p2q = pp[("q", hf)]
                    for sch in range(TCH):
                        lhsT = kr[hf][:, cc * T + sch * P: cc * T + (sch + 1) * P]
                        nc.tensor.matmul(sc_ps[sch][:], lhsT,
                                         p1q[:, cc * T:(cc + 1) * T],
                                         start=(c == 0), stop=False)
                        nc.tensor.matmul(sc_ps[sch][:], lhsT,
                                         p2q[:, cc * T:(cc + 1) * T],
                                         start=False, stop=(c == NCH - 1))
                sc_sb = sc_pool.tile([P, TCH * T], mybir.dt.bfloat16, tag="scsb")
                for sch in range(TCH):
                    nc.scalar.copy(sc_sb[:, sch * T:(sch + 1) * T], sc_ps[sch][:])

                # mm2: out[t_chunk] [128t, 256d] = sum_s scoresT[s][:,t_chunk] @ V[s]
                o_ps = ps_out.tile([P, TCH * D], mybir.dt.float32, tag="ops")
                for tch in range(TCH):
                    for sch in range(TCH):
                        lhsT = sc_sb[:, sch * T + tch * P: sch * T + (tch + 1) * P]
                        rhs = v_bf[:, sch * D:(sch + 1) * D]
                        nc.tensor.matmul(o_ps[:, tch * D:(tch + 1) * D], lhsT, rhs,
                                         start=(sch == 0), stop=(sch == TCH - 1))
                o_sb = out_pool.tile([P, TCH * D], mybir.dt.bfloat16, tag="osb")
                nc.scalar.copy(o_sb[:], o_ps[:])
                nc.scalar.dma_start(o_dram[b, h], o_sb[:])
    return nc


_NC_CACHE = None


def _stage_qk(x):
    """[B,H,T,N] f32 -> bf16, pi order, [B,H,2,128,2048] half-tile images."""
    xb = x.astype(bf16)
    xt = xb.transpose(0, 1, 3, 2)                 # [B,H,N,T] view
    out = np.empty((x.shape[0], x.shape[1], 2, P, HW_), dtype=bf16)
    for hf, sl in ((0, xt[:, :, 0::2, :]), (1, xt[:, :, 1::2, :])):
        # [B,H,1024,T] -> [B,H,8,128,T] -> [B,H,128,8,T] -> [B,H,128,2048]
        out[:, :, hf] = np.ascontiguousarray(
            sl.reshape(x.shape[0], x.shape[1], HC, P, T).transpose(0, 1, 3, 2, 4)
        ).reshape(x.shape[0], x.shape[1], P, HW_)
    return out


def _stage_v(V):
    """[B,1,T,D] f32 -> bf16 tile image [B,128,TCH*D]."""
    vb = V.astype(bf16)[:, 0]                     # [B,T,D]
    return np.ascontiguousarray(
        vb.reshape(B, TCH, P, D).transpose(0, 2, 1, 3)).reshape(B, P, TCH * D)


def _unstage_o(o):
    """[B,H,128,TCH*D] bf16 tile image -> [B,H,T,D] f32."""
    return np.ascontiguousarray(
        o.reshape(B, H, P, TCH, D).transpose(0, 1, 3, 2, 4)
    ).reshape(B, H, T, D).astype(np.float32)


def kernel(Q, K, V):
    global _NC_CACHE, LAST_RESULT
    Q = np.asarray(Q, dtype=np.float32)
    K = np.asarray(K, dtype=np.float32)
    V = np.asarray(V, dtype=np.float32)
    assert Q.shape == (B, H, T, N) and K.shape == (B, H, T, N) and V.shape == (B, 1, T, D)

    if _NC_CACHE is None:
        _NC_CACHE = _build_nc()
        _NC_CACHE.compile()
    nc = _NC_CACHE
    ct, st = _rope_tables()
    QT = _stage_qk(Q)
    KT = _stage_qk(K)
    Vb = _stage_v(V)

    in_maps = []
    for c in range(NCORES):
        sl = slice(c * BS, (c + 1) * BS)
        in_maps.append({
            "QT": np.ascontiguousarray(QT[sl]),
            "KT": np.ascontiguousarray(KT[sl]),
            "V": np.ascontiguousarray(Vb[sl]),
            "CT": ct, "ST": st,
        })

    trace = bool(os.environ.get("BASS_KERNEL_TRACE"))
    res = run_bass_kernel_spmd(nc, in_maps, list(range(NCORES)), trace=trace,
                               trace_cores=[0] if trace else None)
    LAST_RESULT = res
    o = np.concatenate([res.results[c]["O"] for c in range(NCORES)], axis=0)
    return _unstage_o(o)
